# revision 8
# baseline (speedup 1.0000x reference)
"""Trainium2 Bass kernel for nn_CrossAttentionLayer (dual cross-attention +
self-attention transformer block), data-parallel over batch on 8 NeuronCores.

Contract: kernel(**inputs) takes the FULL unsharded inputs (as produced by
setup_inputs()) and returns the full (text_att, image_att) tuple.
"""

import sys

sys.path.insert(0, "/opt/trn_rl_repo")

import numpy as np

# Problem shapes (hardcoded per harness contract).
B, LT, LI, DT, DI, D, H = 128, 96, 64, 1024, 2048, 1024, 4
DH = D // H  # 256 per head
NCORES = 8
BC = B // NCORES  # 16 batch elements per core
EPS = 1e-3
NEGBIG = -1.0e9

_CACHE = {}


def _build(bc):
    import concourse.bass as bass
    from concourse import bacc
    import concourse.mybir as mybir
    import concourse.tile as tile
    from concourse.masks import make_identity

    f32 = mybir.dt.float32
    f32r = mybir.dt.float32r
    AF = mybir.ActivationFunctionType
    AX = mybir.AxisListType.X
    ALU = mybir.AluOpType

    NT = bc * LT  # text tokens per core
    NI = bc * LI  # image tokens per core
    KT = DT // 128  # 8
    KI = DI // 128  # 16
    KD = D // 128  # 8
    MD = D // 128  # 8

    nc = bacc.Bacc("TRN2", target_bir_lowering=False, debug=False)

    # ---------------- external I/O ----------------
    xt = nc.declare_dram_parameter("xt", [NT, DT], f32r, isOutput=False)
    xi = nc.declare_dram_parameter("xi", [NI, DI], f32r, isOutput=False)
    # additive key masks (-1e9 where masked, 0 where kept)
    am_t = nc.declare_dram_parameter("am_t", [bc, LT], f32r, isOutput=False)
    am_i = nc.declare_dram_parameter("am_i", [bc, LI], f32r, isOutput=False)
    # row-valid multipliers (qmask * any(kmask)) per attention
    rv_ta = nc.declare_dram_parameter("rv_ta", [bc, LI], f32, isOutput=False)
    rv_ia = nc.declare_dram_parameter("rv_ia", [bc, LT], f32, isOutput=False)
    rv_ts = nc.declare_dram_parameter("rv_ts", [bc, LI], f32, isOutput=False)
    rv_is = nc.declare_dram_parameter("rv_is", [bc, LT], f32, isOutput=False)

    wnames = [
        ("w_tp", DT, D), ("w_ip", DI, D),
        ("ta_wq", D, D), ("ta_wk", D, D), ("ta_wv", DT, D), ("ta_wo", D, D),
        ("ia_wq", D, D), ("ia_wk", D, D), ("ia_wv", DI, D), ("ia_wo", D, D),
        ("ts_wq", D, D), ("ts_wk", D, D), ("ts_wv", D, D), ("ts_wo", D, D),
        ("is_wq", D, D), ("is_wk", D, D), ("is_wv", D, D), ("is_wo", D, D),
    ]
    W = {n: nc.declare_dram_parameter(n, [k, m], f32r, isOutput=False)
         for n, k, m in wnames}
    vnames = ["b_tp", "b_ip", "g_ta", "bt_ta", "g_ia", "bt_ia",
              "g_ts", "bt_ts", "g_is", "bt_is"]
    V1 = {n: nc.declare_dram_parameter(n, [D], f32, isOutput=False)
          for n in vnames}

    out_t = nc.declare_dram_parameter("out_t", [NI, D], f32, isOutput=True)
    out_i = nc.declare_dram_parameter("out_i", [NT, D], f32, isOutput=True)

    # ---------------- DRAM intermediates ----------------
    xtT = nc.dram_tensor("xtT", [DT, NT], f32r)
    xiT = nc.dram_tensor("xiT", [DI, NI], f32r)
    tpT = nc.dram_tensor("tpT", [D, NT], f32r)   # text_proj^T
    ipT = nc.dram_tensor("ipT", [D, NI], f32r)   # image_proj^T
    v_ta = nc.dram_tensor("v_ta", [bc, LT, D], f32r)       # per-batch V (text keys)
    v_ia = nc.dram_tensor("v_ia", [bc // 2, 2 * LI, D], f32r)  # batch-pair V
    att1tT = nc.dram_tensor("att1tT", [D, NI], f32r)  # ln(cross text_att)^T
    att1iT = nc.dram_tensor("att1iT", [D, NT], f32r)  # ln(cross image_att)^T
    v_ts = nc.dram_tensor("v_ts", [bc // 2, 2 * LI, D], f32r)
    v_is = nc.dram_tensor("v_is", [bc, LT, D], f32r)
    qtsT = nc.dram_tensor("qtsT", [D, NI], f32r)
    ktsT = nc.dram_tensor("ktsT", [D, NI], f32r)
    qisT = nc.dram_tensor("qisT", [D, NT], f32r)
    kisT = nc.dram_tensor("kisT", [D, NT], f32r)

    with tile.TileContext(nc) as tc:
        # ---------- persistent small constants ----------
        const_pool = tc.tile_pool(name="consts", bufs=1)
        cp = const_pool.__enter__()
        ident_f = cp.tile([128, 128], f32, tag="idf")
        make_identity(nc, ident_f)
        ident = cp.tile([128, 128], f32r, tag="idr")
        nc.vector.tensor_copy(ident[:], ident_f[:])
        ones_row_f = cp.tile([1, 128], f32, tag="o1rf")
        nc.vector.memset(ones_row_f[:], 1.0)
        ones_row = cp.tile([1, 128], f32r, tag="o1r")
        nc.vector.tensor_copy(ones_row[:], ones_row_f[:])
        ones_col_f = cp.tile([128, 1], f32, tag="o1cf")
        nc.vector.memset(ones_col_f[:], 1.0)
        ones_col = cp.tile([128, 1], f32r, tag="o1c")
        nc.vector.tensor_copy(ones_col[:], ones_col_f[:])
        eps_col = cp.tile([128, 1], f32, tag="epsc")
        nc.vector.memset(eps_col[:], EPS)

        def load_pc_vec(pool, dram_vec, tag):
            """[D] f32 dram -> [128, D//128] sbuf (feature chunk layout)."""
            t = pool.tile([128, D // 128], f32, tag=tag)
            with nc.allow_non_contiguous_dma(reason="tiny aux vector"):
                nc.sync.dma_start(t[:], dram_vec.rearrange("(c p) -> p c", p=128))
            return t

        # =========================================================
        # Phase 0: transpose raw embeddings -> xtT, xiT  (feature-major)
        # =========================================================
        def transpose_to_dram(src, srcT, ntok, dfeat):
            with tc.tile_pool(name="tr_in", bufs=3) as tin, \
                 tc.tile_pool(name="tr_out", bufs=4) as tout, \
                 tc.tile_pool(name="tr_ps", bufs=4, space="PSUM") as tps:
                for to in range(ntok // 128):
                    it = tin.tile([128, dfeat], f32r, tag="in")
                    nc.sync.dma_start(it[:], src[to * 128:(to + 1) * 128, :])
                    for fo in range(dfeat // 128):
                        ps = tps.tile([128, 128], f32r, tag="ps")
                        nc.tensor.transpose(ps[:], it[:, fo * 128:(fo + 1) * 128],
                                            ident[:])
                        ot = tout.tile([128, 128], f32r, tag="ot")
                        nc.vector.tensor_copy(ot[:], ps[:])
                        nc.sync.dma_start(
                            srcT[fo * 128:(fo + 1) * 128,
                                 to * 128:(to + 1) * 128], ot[:])

        transpose_to_dram(xt, xtT, NT, DT)
        transpose_to_dram(xi, xiT, NI, DI)

        # =========================================================
        # generic projections
        # =========================================================
        def proj_featmaj(w_dram, din, dout, src_rhs, ntok, out_write,
                         bias_sb=None):
            """out^T[dout, ntok] = W^T @ X^T. src_rhs(ko, no, nsz) -> rhs AP
            [128, nsz]; out_write(mo, no, nsz, psum_ap) emits copyback."""
            KC = din // 128
            MC = dout // 128
            NO = (ntok + 511) // 512
            with tc.tile_pool(name="pw", bufs=1) as pw, \
                 tc.tile_pool(name="ppsum", bufs=4, space="PSUM") as pp:
                w_sb = pw.tile([128, KC, dout], f32r, tag="w")
                nc.sync.dma_start(
                    w_sb[:], w_dram.rearrange("(kc p) n -> p kc n", p=128))
                for no in range(NO):
                    nsz = min(512, ntok - no * 512)
                    rhs = [src_rhs(ko, no, nsz) for ko in range(KC)]
                    for mo in range(MC):
                        ps = pp.tile([128, 512], f32, tag="ps")
                        for ko in range(KC):
                            nc.tensor.matmul(
                                ps[:, :nsz],
                                w_sb[:, ko, mo * 128:(mo + 1) * 128],
                                rhs[ko][:, :nsz],
                                start=(ko == 0), stop=(ko == KC - 1))
                        out_write(mo, no, nsz, ps)

        def dram_rhs_loader(pool, srcT, tag):
            def f(ko, no, nsz):
                t = pool.tile([128, 512], f32r, tag=tag)
                nc.sync.dma_start(t[:, :nsz],
                                  srcT[ko * 128:(ko + 1) * 128,
                                       no * 512:no * 512 + nsz])
                return t
            return f

        def proj_tokmaj(w_dram, din, dout, lhsT_src, tok_tiles, out_write):
            """out[tok, dout] = X @ W. lhsT_src(ti, ko) -> [128, rows<=128]
            stationary AP; out_write(ti, no, nsz, rows, psum)."""
            KC = din // 128
            NO = dout // 512
            with tc.tile_pool(name="pw2", bufs=1) as pw, \
                 tc.tile_pool(name="ppsum2", bufs=4, space="PSUM") as pp:
                w_sb = pw.tile([128, KC, dout], f32r, tag="w")
                nc.sync.dma_start(
                    w_sb[:], w_dram.rearrange("(kc p) n -> p kc n", p=128))
                for ti, rows in tok_tiles:
                    for no in range(NO):
                        nsz = 512
                        ps = pp.tile([128, 512], f32, tag="ps")
                        for ko in range(KC):
                            nc.tensor.matmul(
                                ps[:rows, :nsz],
                                lhsT_src(ti, ko),
                                w_sb[:, ko, no * 512:no * 512 + nsz],
                                start=(ko == 0), stop=(ko == KC - 1))
                        out_write(ti, no, nsz, rows, ps)

        # =========================================================
        # Phase 1: text_proj^T, image_proj^T  (with bias), into DRAM
        # =========================================================
        with tc.tile_pool(name="p1aux", bufs=1) as p1aux, \
             tc.tile_pool(name="p1rhs", bufs=18) as p1rhs, \
             tc.tile_pool(name="p1out", bufs=4) as p1out:
            btp = load_pc_vec(p1aux, V1["b_tp"], "btp")

            def wr_tp(mo, no, nsz, ps):
                o = p1out.tile([128, 512], f32r, tag="o")
                nc.vector.tensor_scalar(o[:, :nsz], ps[:, :nsz],
                                        btp[:, mo:mo + 1], None, ALU.add)
                nc.sync.dma_start(
                    tpT[mo * 128:(mo + 1) * 128, no * 512:no * 512 + nsz],
                    o[:, :nsz])
            proj_featmaj(W["w_tp"], DT, D, dram_rhs_loader(p1rhs, xtT, "x"),
                         NT, wr_tp)

            bip = load_pc_vec(p1aux, V1["b_ip"], "bip")

            def wr_ip(mo, no, nsz, ps):
                o = p1out.tile([128, 512], f32r, tag="o")
                nc.vector.tensor_scalar(o[:, :nsz], ps[:, :nsz],
                                        bip[:, mo:mo + 1], None, ALU.add)
                nc.sync.dma_start(
                    ipT[mo * 128:(mo + 1) * 128, no * 512:no * 512 + nsz],
                    o[:, :nsz])
            proj_featmaj(W["w_ip"], DI, D, dram_rhs_loader(p1rhs, xiT, "x2"),
                         NI, wr_ip)

        # =========================================================
        # Phase 2: value projections (token-major, per-batch tiles)
        # =========================================================
        def v_proj(w_dram, din, xT_dram, vd, tiles_spec):
            # tiles_spec: list of (ti, rows, colslice_start)
            with tc.tile_pool(name="vx", bufs=4) as vx, \
                 tc.tile_pool(name="vo", bufs=4) as vo:
                KC = din // 128
                xcols = {}

                def lhsT_src(ti, ko):
                    if ti not in xcols:
                        rows = tiles_spec[ti][1]
                        cs = tiles_spec[ti][2]
                        t = vx.tile([128, KC, 128], f32r, tag="xc")
                        nc.sync.dma_start(
                            t[:, :, :rows],
                            xT_dram[:, cs:cs + rows].rearrange(
                                "(kc p) t -> p kc t", p=128))
                        xcols[ti] = t
                        if len(xcols) > 3:
                            xcols.pop(next(iter(xcols)))
                    rows = tiles_spec[ti][1]
                    return xcols[ti][:, ko, :rows]

                def out_write(ti, no, nsz, rows, ps):
                    o = vo.tile([128, 512], f32r, tag="o")
                    nc.vector.tensor_copy(o[:rows, :nsz], ps[:rows, :nsz])
                    nc.sync.dma_start(
                        vd[ti, 0:rows, no * 512:no * 512 + nsz],
                        o[:rows, :nsz])

                proj_tokmaj(w_dram, din, D, lhsT_src,
                            [(ti, rows) for ti, rows, _ in tiles_spec],
                            out_write)

        v_proj(W["ta_wv"], DT, xtT, v_ta,
               [(b, LT, b * LT) for b in range(bc)])
        v_proj(W["ia_wv"], DI, xiT, v_ia,
               [(p, 128, p * 128) for p in range(bc // 2)])

        # =========================================================
        # attention block
        # =========================================================
        def attention(qT_sb, kT_sb, vd, Lq, Lk, pair_mode, am_dram, rv_dram,
                      oT_sb, ntokq):
            """qT_sb/kT_sb: [128, KD, ntok] sbuf; vd: dram V; oT_sb out."""
            with tc.tile_pool(name="at_v", bufs=3) as atv, \
                 tc.tile_pool(name="at_am", bufs=4) as atam, \
                 tc.tile_pool(name="at_sm", bufs=6) as atsm, \
                 tc.tile_pool(name="at_aT", bufs=4) as ataT, \
                 tc.tile_pool(name="at_ps", bufs=2, space="PSUM") as sps, \
                 tc.tile_pool(name="at_pt", bufs=2, space="PSUM") as tps, \
                 tc.tile_pool(name="at_pv", bufs=4, space="PSUM") as vps, \
                 tc.tile_pool(name="at_rv", bufs=1) as atrv:
                rvt = atrv.tile([128, bc], f32, tag="rv")
                with nc.allow_non_contiguous_dma(reason="tiny rowvalid"):
                    nc.sync.dma_start(rvt[:Lq, :],
                                      rv_dram.rearrange("b q -> q b"))
                vrows = 2 * Lk if pair_mode else Lk
                for b in range(bc):
                    if pair_mode:
                        if b % 2 == 0:
                            vtile = atv.tile([128, D], f32r, tag="v")
                            nc.sync.dma_start(vtile[:], vd[b // 2])
                        vbase = (b % 2) * Lk
                    else:
                        vtile = atv.tile([Lk, D], f32r, tag="v")
                        nc.sync.dma_start(vtile[:], vd[b])
                        vbase = 0
                    amr = atam.tile([1, Lk], f32r, tag="am")
                    nc.sync.dma_start(amr[:], am_dram[b:b + 1, :])
                    for h in range(H):
                        S = sps.tile([Lq, Lk], f32, tag="S")
                        for c in range(2):
                            nc.tensor.matmul(
                                S[:],
                                qT_sb[:, 2 * h + c, b * Lq:(b + 1) * Lq],
                                kT_sb[:, 2 * h + c, b * Lk:(b + 1) * Lk],
                                start=(c == 0), stop=False)
                        nc.tensor.matmul(S[:], ones_row[:, :Lq], amr[:],
                                         start=False, stop=True)
                        negmax = atsm.tile([Lq, 1], f32, tag="nm")
                        nc.vector.reduce_max(negmax[:], S[:], AX)
                        nc.vector.tensor_scalar_mul(negmax[:], negmax[:],
                                                    -1.0 / 16.0)
                        P = atsm.tile([Lq, Lk], f32, tag="P")
                        denom = atsm.tile([Lq, 1], f32, tag="dn")
                        nc.scalar.activation(P[:], S[:], AF.Exp,
                                             bias=negmax[:], scale=1.0 / 16.0,
                                             accum_out=denom[:])
                        rsc = atsm.tile([Lq, 1], f32, tag="rs")
                        nc.vector.reciprocal(rsc[:], denom[:])
                        nc.vector.tensor_mul(rsc[:], rsc[:],
                                             rvt[:Lq, b:b + 1])
                        Pn = atsm.tile([Lq, Lk], f32r, tag="Pn")
                        nc.vector.tensor_scalar_mul(Pn[:], P[:], rsc[:])
                        aTp = tps.tile([Lk, Lq], f32r, tag="aT")
                        nc.tensor.transpose(aTp[:], Pn[:], ident[:Lq, :Lq])
                        aT = ataT.tile([128, Lq], f32r, tag="aTs")
                        nc.vector.tensor_copy(aT[vbase:vbase + Lk, :], aTp[:])
                        for dvs in range(2):
                            fo = 2 * h + dvs
                            pv = vps.tile([128, Lq], f32, tag="pv")
                            nc.tensor.matmul(
                                pv[:],
                                vtile[vbase:vbase + Lk,
                                      fo * 128:(fo + 1) * 128],
                                aT[vbase:vbase + Lk, :],
                                start=True, stop=True)
                            nc.vector.tensor_copy(
                                oT_sb[:, fo, b * Lq:(b + 1) * Lq], pv[:])

        # =========================================================
        # LN helpers
        # =========================================================
        def ln_featmaj(x_sb, ntok, g_sb, b_sb, outT_dram):
            """x_sb: [128, KD, ntok] f32r feature-major (raw).
            Writes normalized f32r to outT_dram."""
            with tc.tile_pool(name="lnsq", bufs=3) as lsq, \
                 tc.tile_pool(name="lnst", bufs=4) as lst, \
                 tc.tile_pool(name="lnps", bufs=2, space="PSUM") as lps, \
                 tc.tile_pool(name="lnrep", bufs=2, space="PSUM") as lrep, \
                 tc.tile_pool(name="lnout", bufs=4) as lout:
                NO = (ntok + 511) // 512
                for no in range(NO):
                    nsz = min(512, ntok - no * 512)
                    sl = slice(no * 512, no * 512 + nsz)
                    psum_t = lps.tile([1, 512], f32, tag="s")
                    psq = lps.tile([1, 512], f32, tag="q")
                    for ko in range(KD):
                        nc.tensor.matmul(psum_t[:, :nsz], ones_col[:],
                                         x_sb[:, ko, sl],
                                         start=(ko == 0), stop=(ko == KD - 1))
                    sqt = None
                    for ko in range(KD):
                        sq = lsq.tile([128, 512], f32r, tag="sq")
                        nc.vector.tensor_mul(sq[:, :nsz], x_sb[:, ko, sl],
                                             x_sb[:, ko, sl])
                        nc.tensor.matmul(psq[:, :nsz], ones_col[:],
                                         sq[:, :nsz],
                                         start=(ko == 0), stop=(ko == KD - 1))
                    stats = lst.tile([1, 1024], f32r, tag="st")
                    mean = lst.tile([1, 512], f32, tag="mn")
                    nc.vector.tensor_scalar_mul(mean[:, :nsz], psum_t[:, :nsz],
                                                1.0 / D)
                    ex2 = lst.tile([1, 512], f32, tag="e2")
                    nc.vector.tensor_scalar_mul(ex2[:, :nsz], psq[:, :nsz],
                                                1.0 / D)
                    msq = lst.tile([1, 512], f32, tag="m2")
                    nc.vector.tensor_mul(msq[:, :nsz], mean[:, :nsz],
                                         mean[:, :nsz])
                    var = lst.tile([1, 512], f32, tag="vr")
                    nc.vector.tensor_sub(var[:, :nsz], ex2[:, :nsz],
                                         msq[:, :nsz])
                    # stats[0:512] = rstd, stats[512:1024] = mean*rstd
                    sd = lst.tile([1, 512], f32, tag="sd")
                    nc.scalar.activation(sd[:, :nsz], var[:, :nsz],
                                         AF.Sqrt, bias=eps_col[:1], scale=1.0)
                    with nc.allow_low_precision(reason="f32r bytes==f32"):
                        nc.vector.reciprocal(stats[:, :nsz], sd[:, :nsz])
                    nc.vector.tensor_mul(stats[:, 512:512 + nsz],
                                         mean[:, :nsz], stats[:, :nsz])
                    rep_r = lrep.tile([128, 512], f32, tag="rr")
                    nc.tensor.matmul(rep_r[:, :nsz], ones_row[:, :],
                                     stats[:, :nsz], start=True, stop=True)
                    rep_m = lrep.tile([128, 512], f32, tag="rm")
                    nc.tensor.matmul(rep_m[:, :nsz], ones_row[:, :],
                                     stats[:, 512:512 + nsz],
                                     start=True, stop=True)
                    for ko in range(KD):
                        o = lout.tile([128, 512], f32r, tag="o")
                        nc.vector.tensor_mul(o[:, :nsz], x_sb[:, ko, sl],
                                             rep_r[:, :nsz])
                        nc.vector.tensor_sub(o[:, :nsz], o[:, :nsz],
                                             rep_m[:, :nsz])
                        nc.vector.tensor_scalar(o[:, :nsz], o[:, :nsz],
                                                g_sb[:, ko:ko + 1],
                                                b_sb[:, ko:ko + 1],
                                                ALU.mult, ALU.add)
                        nc.sync.dma_start(
                            outT_dram[ko * 128:(ko + 1) * 128, sl],
                            o[:, :nsz])

        def ln_tokmaj_out(ps_getter, tok_tiles, g_sb_row, b_sb_row, out_dram):
            """Final LN on token-major tiles; ps_getter(ti) -> sbuf [rows, D]
            f32 tile. Writes f32 out_dram rows."""
            with tc.tile_pool(name="lnf", bufs=4) as lnf:
                for ti, rows, rstart in tok_tiles:
                    x = ps_getter(ti, rows)
                    mean = lnf.tile([128, 1], f32, tag="mn")
                    nc.vector.reduce_sum(mean[:rows], x[:rows], AX)
                    nc.vector.tensor_scalar_mul(mean[:rows], mean[:rows],
                                                1.0 / D)
                    nc.vector.tensor_scalar(x[:rows], x[:rows], mean[:rows],
                                            None, ALU.subtract)
                    sq = lnf.tile([128, D], f32, tag="sq")
                    nc.vector.tensor_mul(sq[:rows], x[:rows], x[:rows])
                    vs = lnf.tile([128, 1], f32, tag="vs")
                    nc.vector.reduce_sum(vs[:rows], sq[:rows], AX)
                    rstd = lnf.tile([128, 1], f32, tag="rs")
                    nc.scalar.activation(rstd[:rows], vs[:rows], AF.Sqrt,
                                         bias=eps_col[:rows], scale=1.0 / D)
                    nc.vector.reciprocal(rstd[:rows], rstd[:rows])
                    nc.vector.tensor_scalar_mul(x[:rows], x[:rows],
                                                rstd[:rows])
                    o = lnf.tile([128, D], f32, tag="o")
                    nc.vector.tensor_mul(o[:rows], x[:rows], g_sb_row[:rows])
                    nc.vector.tensor_add(o[:rows], o[:rows], b_sb_row[:rows])
                    nc.sync.dma_start(out_dram[rstart:rstart + rows, :],
                                      o[:rows])

        # =========================================================
        # cross-attention pipelines
        # =========================================================
        def cross_side(wq, wk, wo, gname, bname, qsrcT, ksrcT, vd, Lq, Lk,
                       pair_mode, am_dram, rv_dram, ntokq, outT_dram):
            po_cm = tc.tile_pool(name="cs_o", bufs=1)
            po = po_cm.__enter__()
            oT = po.tile([128, KD, ntokq], f32r, tag="oT")
            pq_cm = tc.tile_pool(name="cs_q", bufs=1)
            pq = pq_cm.__enter__()
            qT = pq.tile([128, KD, ntokq], f32r, tag="qT")
            kT = pq.tile([128, KD, bc * Lk], f32r, tag="kT")

            prhs_cm = tc.tile_pool(name="cs_rhs", bufs=10)
            prhs = prhs_cm.__enter__()

            def wr_q(mo, no, nsz, ps):
                nc.vector.tensor_copy(
                    qT[:, mo, no * 512:no * 512 + nsz], ps[:, :nsz])
            proj_featmaj(W[wq], D, D, dram_rhs_loader(prhs, qsrcT, "xq"),
                         ntokq, wr_q)

            def wr_k(mo, no, nsz, ps):
                nc.vector.tensor_copy(
                    kT[:, mo, no * 512:no * 512 + nsz], ps[:, :nsz])
            proj_featmaj(W[wk], D, D, dram_rhs_loader(prhs, ksrcT, "xk"),
                         bc * Lk, wr_k)
            prhs_cm.__exit__(None, None, None)

            attention(qT, kT, vd, Lq, Lk, pair_mode, am_dram,
                      rv_dram, oT, ntokq)
            pq_cm.__exit__(None, None, None)

            # Wo projection (feature-major out) + LN -> DRAM
            with tc.tile_pool(name="cs_w1", bufs=1) as pw1, \
                 tc.tile_pool(name="cs_ln", bufs=1) as pln:
                araw = pw1.tile([128, KD, ntokq], f32r, tag="ar")

                def wr_a(mo, no, nsz, ps):
                    nc.vector.tensor_copy(
                        araw[:, mo, no * 512:no * 512 + nsz],
                        ps[:, :nsz])

                def orhs(ko, no, nsz):
                    return oT[:, ko, no * 512:no * 512 + nsz]
                proj_featmaj(W[wo], D, D, orhs, ntokq, wr_a)
                g_sb = load_pc_vec(pln, V1[gname], "g")
                b_sb = load_pc_vec(pln, V1[bname], "b")
                ln_featmaj(araw, ntokq, g_sb, b_sb, outT_dram)
            po_cm.__exit__(None, None, None)

        # ta: queries = image side (Lq=LI), keys/values = text side
        cross_side("ta_wq", "ta_wk", "ta_wo", "g_ta", "bt_ta",
                   ipT, tpT, v_ta, LI, LT, False, am_t, rv_ta, NI, att1tT)
        # ia: queries = text side, keys/values = image side (pair mode)
        cross_side("ia_wq", "ia_wk", "ia_wo", "g_ia", "bt_ia",
                   tpT, ipT, v_ia, LT, LI, True, am_i, rv_ia, NT, att1iT)

        # =========================================================
        # self-attention QKV projections (from att1*T)
        # =========================================================
        with tc.tile_pool(name="sa_rhs", bufs=18) as sarhs, \
             tc.tile_pool(name="sa_out", bufs=4) as saout:
            def dram_writer(dst):
                def wr(mo, no, nsz, ps):
                    o = saout.tile([128, 512], f32r, tag="o")
                    nc.vector.tensor_copy(o[:, :nsz], ps[:, :nsz])
                    nc.sync.dma_start(
                        dst[mo * 128:(mo + 1) * 128, no * 512:no * 512 + nsz],
                        o[:, :nsz])
                return wr
            proj_featmaj(W["ts_wq"], D, D,
                         dram_rhs_loader(sarhs, att1tT, "x1"), NI,
                         dram_writer(qtsT))
            proj_featmaj(W["ts_wk"], D, D,
                         dram_rhs_loader(sarhs, att1tT, "x2"), NI,
                         dram_writer(ktsT))
            proj_featmaj(W["is_wq"], D, D,
                         dram_rhs_loader(sarhs, att1iT, "x3"), NT,
                         dram_writer(qisT))
            proj_featmaj(W["is_wk"], D, D,
                         dram_rhs_loader(sarhs, att1iT, "x4"), NT,
                         dram_writer(kisT))
        v_proj(W["ts_wv"], D, att1tT, v_ts,
               [(p, 128, p * 128) for p in range(bc // 2)])
        v_proj(W["is_wv"], D, att1iT, v_is,
               [(b, LT, b * LT) for b in range(bc)])

        # =========================================================
        # self-attention + final Wo (token-major) + final LN -> outputs
        # =========================================================
        def self_side(qTd, kTd, vd, Lqk, pair_mode, am_dram, rv_dram, ntok,
                      wo, gname, bname, out_dram):
            po_cm = tc.tile_pool(name="ss_o", bufs=1)
            po = po_cm.__enter__()
            oT = po.tile([128, KD, ntok], f32r, tag="oT")
            pq_cm = tc.tile_pool(name="ss_q", bufs=1)
            pq = pq_cm.__enter__()
            qT = pq.tile([128, KD, ntok], f32r, tag="qT")
            kT = pq.tile([128, KD, ntok], f32r, tag="kT")
            for buf, src_ in ((qT, qTd), (kT, kTd)):
                for ko in range(KD):
                    for no in range((ntok + 511) // 512):
                        nsz = min(512, ntok - no * 512)
                        nc.sync.dma_start(
                            buf[:, ko, no * 512:no * 512 + nsz],
                            src_[ko * 128:(ko + 1) * 128,
                                 no * 512:no * 512 + nsz])
            attention(qT, kT, vd, Lqk, Lqk, pair_mode, am_dram,
                      rv_dram, oT, ntok)
            pq_cm.__exit__(None, None, None)

            # final Wo token-major + LN
            with tc.tile_pool(name="ss_w", bufs=1) as pw, \
                 tc.tile_pool(name="ss_x", bufs=3) as px, \
                 tc.tile_pool(name="ss_ps", bufs=4,
                              space="PSUM") as pp, \
                 tc.tile_pool(name="ss_ln", bufs=1) as pln:
                w_sb = pw.tile([128, KD, D], f32r, tag="w")
                nc.sync.dma_start(
                    w_sb[:],
                    W[wo].rearrange("(kc p) n -> p kc n", p=128))
                # replicate g/b rows across partitions via PE
                grow = pln.tile([1, D], f32r, tag="gr")
                brow = pln.tile([1, D], f32r, tag="br")
                nc.sync.dma_start(grow[:], V1[gname][None, :].bitcast(f32r))
                nc.sync.dma_start(brow[:], V1[bname][None, :].bitcast(f32r))
                g_rep = pln.tile([128, D], f32, tag="grep")
                b_rep = pln.tile([128, D], f32, tag="brep")
                for half in range(2):
                    hs = slice(half * 512, (half + 1) * 512)
                    rp = pp.tile([128, 512], f32, tag="rep")
                    nc.tensor.matmul(rp[:], ones_row[:], grow[:, hs],
                                     start=True, stop=True)
                    nc.vector.tensor_copy(g_rep[:, hs], rp[:])
                    rp2 = pp.tile([128, 512], f32, tag="rep")
                    nc.tensor.matmul(rp2[:], ones_row[:], brow[:, hs],
                                     start=True, stop=True)
                    nc.vector.tensor_copy(b_rep[:, hs], rp2[:])

                x_tiles = {}

                def getx(ti, rows):
                    return x_tiles.pop(ti)

                for ti in range(ntok // 128):
                    xsb = px.tile([128, D], f32, tag="x")
                    for no in range(2):
                        ps = pp.tile([128, 512], f32, tag="ps")
                        for ko in range(KD):
                            nc.tensor.matmul(
                                ps[:],
                                oT[:, ko, ti * 128:(ti + 1) * 128],
                                w_sb[:, ko, no * 512:(no + 1) * 512],
                                start=(ko == 0), stop=(ko == KD - 1))
                        nc.vector.tensor_copy(
                            xsb[:, no * 512:(no + 1) * 512], ps[:])
                    x_tiles[ti] = xsb
                    ln_tokmaj_out(getx,
                                  [(ti, 128, ti * 128)],
                                  g_rep, b_rep, out_dram)
            po_cm.__exit__(None, None, None)

        self_side(qtsT, ktsT, v_ts, LI, True, am_i, rv_ts, NI,
                  "ts_wo", "g_ts", "bt_ts", out_t)
        self_side(qisT, kisT, v_is, LT, False, am_t, rv_is, NT,
                  "is_wo", "g_is", "bt_is", out_i)

        const_pool.__exit__(None, None, None)

    nc.compile()
    return nc


def _get_nc(bc):
    if bc not in _CACHE:
        _CACHE[bc] = _build(bc)
    return _CACHE[bc]


def kernel(text_embedding, image_embedding, text_mask, image_mask,
           W_tp, b_tp, W_ip, b_ip,
           ta_Wq, ta_Wk, ta_Wv, ta_Wo,
           ia_Wq, ia_Wk, ia_Wv, ia_Wo,
           ts_Wq, ts_Wk, ts_Wv, ts_Wo,
           is_Wq, is_Wk, is_Wv, is_Wo,
           ln_ta_g, ln_ta_b, ln_ia_g, ln_ia_b,
           ln_ts_g, ln_ts_b, ln_is_g, ln_is_b):
    from concourse.bass_utils import run_bass_kernel_spmd

    bc = B // NCORES
    nc = _get_nc(bc)

    f = np.float32
    wmap = {
        "w_tp": W_tp, "w_ip": W_ip,
        "ta_wq": ta_Wq, "ta_wk": ta_Wk, "ta_wv": ta_Wv, "ta_wo": ta_Wo,
        "ia_wq": ia_Wq, "ia_wk": ia_Wk, "ia_wv": ia_Wv, "ia_wo": ia_Wo,
        "ts_wq": ts_Wq, "ts_wk": ts_Wk, "ts_wv": ts_Wv, "ts_wo": ts_Wo,
        "is_wq": is_Wq, "is_wk": is_Wk, "is_wv": is_Wv, "is_wo": is_Wo,
        "b_tp": b_tp, "b_ip": b_ip,
        "g_ta": ln_ta_g, "bt_ta": ln_ta_b, "g_ia": ln_ia_g, "bt_ia": ln_ia_b,
        "g_ts": ln_ts_g, "bt_ts": ln_ts_b, "g_is": ln_is_g, "bt_is": ln_is_b,
    }
    wmap = {k: np.ascontiguousarray(np.asarray(v), dtype=f)
            for k, v in wmap.items()}

    in_maps = []
    for c in range(NCORES):
        sl = slice(c * bc, (c + 1) * bc)
        tm = np.asarray(text_mask[sl]).astype(f)
        im = np.asarray(image_mask[sl]).astype(f)
        m = dict(wmap)
        m["xt"] = np.ascontiguousarray(
            np.asarray(text_embedding[sl]), dtype=f).reshape(bc * LT, DT)
        m["xi"] = np.ascontiguousarray(
            np.asarray(image_embedding[sl]), dtype=f).reshape(bc * LI, DI)
        m["am_t"] = (NEGBIG * (1.0 - tm)).astype(f)
        m["am_i"] = (NEGBIG * (1.0 - im)).astype(f)
        any_t = (tm.max(axis=1) > 0).astype(f)[:, None]
        any_i = (im.max(axis=1) > 0).astype(f)[:, None]
        m["rv_ta"] = (im * any_t).astype(f)
        m["rv_ia"] = (tm * any_i).astype(f)
        m["rv_ts"] = (im * any_i).astype(f)
        m["rv_is"] = (tm * any_t).astype(f)
        in_maps.append(m)

    res = run_bass_kernel_spmd(nc, in_maps, list(range(NCORES)))
    text = np.concatenate(
        [res.results[c]["out_t"].reshape(bc, LI, D) for c in range(NCORES)],
        axis=0)
    image = np.concatenate(
        [res.results[c]["out_i"].reshape(bc, LT, D) for c in range(NCORES)],
        axis=0)
    return text, image


# revision 14
# speedup vs baseline: 1.0844x; 1.0844x over previous
"""Trainium2 Bass kernel for nn_CrossAttentionLayer (dual cross-attention +
self-attention transformer block), data-parallel over batch on 8 NeuronCores.

Contract: kernel(**inputs) takes the FULL unsharded inputs (as produced by
setup_inputs()) and returns the full (text_att, image_att) tuple.
"""

import sys

sys.path.insert(0, "/opt/trn_rl_repo")

import numpy as np

# Problem shapes (hardcoded per harness contract).
B, LT, LI, DT, DI, D, H = 128, 96, 64, 1024, 2048, 1024, 4
DH = D // H  # 256 per head
NCORES = 8
BC = B // NCORES  # 16 batch elements per core
EPS = 1e-3
NEGBIG = -1.0e9

_CACHE = {}
import os as _os
PACK_MODE = int(_os.environ.get("PACK_MODE", "3"))  # 1=cross-ta, 2=self-ts bits


def _build(bc):
    import concourse.bass as bass
    from concourse import bacc
    import concourse.mybir as mybir
    import concourse.tile as tile
    from concourse.masks import make_identity

    f32 = mybir.dt.float32
    f32r = mybir.dt.float32r
    AF = mybir.ActivationFunctionType
    AX = mybir.AxisListType.X
    ALU = mybir.AluOpType

    NT = bc * LT  # text tokens per core
    NI = bc * LI  # image tokens per core
    KT = DT // 128  # 8
    KI = DI // 128  # 16
    KD = D // 128  # 8
    MD = D // 128  # 8

    nc = bacc.Bacc("TRN2", target_bir_lowering=False, debug=False)

    # ---------------- external I/O ----------------
    xt = nc.declare_dram_parameter("xt", [NT, DT], f32r, isOutput=False)
    xi = nc.declare_dram_parameter("xi", [NI, DI], f32r, isOutput=False)
    # additive key masks (-1e9 where masked, 0 where kept)
    am_t = nc.declare_dram_parameter("am_t", [bc, LT], f32r, isOutput=False)
    am_i = nc.declare_dram_parameter("am_i", [bc, LI], f32r, isOutput=False)
    # row-valid multipliers (qmask * any(kmask)) per attention
    rv_ta = nc.declare_dram_parameter("rv_ta", [bc, LI], f32, isOutput=False)
    rv_ia = nc.declare_dram_parameter("rv_ia", [bc, LT], f32, isOutput=False)
    rv_ts = nc.declare_dram_parameter("rv_ts", [bc, LI], f32, isOutput=False)
    rv_is = nc.declare_dram_parameter("rv_is", [bc, LT], f32, isOutput=False)

    wnames = [
        ("w_tp", DT, D), ("w_ip", DI, D),
        ("ta_wq", D, D), ("ta_wk", D, D), ("ta_wv", DT, D), ("ta_wo", D, D),
        ("ia_wq", D, D), ("ia_wk", D, D), ("ia_wv", DI, D), ("ia_wo", D, D),
        ("ts_wq", D, D), ("ts_wk", D, D), ("ts_wv", D, D), ("ts_wo", D, D),
        ("is_wq", D, D), ("is_wk", D, D), ("is_wv", D, D), ("is_wo", D, D),
    ]
    W = {n: nc.declare_dram_parameter(n, [k, m], f32r, isOutput=False)
         for n, k, m in wnames}
    vnames = ["b_tp", "b_ip", "g_ta", "bt_ta", "g_ia", "bt_ia",
              "g_ts", "bt_ts", "g_is", "bt_is"]
    V1 = {n: nc.declare_dram_parameter(n, [D], f32, isOutput=False)
          for n in vnames}

    out_t = nc.declare_dram_parameter("out_t", [NI, D], f32, isOutput=True)
    out_i = nc.declare_dram_parameter("out_i", [NT, D], f32, isOutput=True)

    # ---------------- DRAM intermediates ----------------
    xtT = nc.dram_tensor("xtT", [DT, NT], f32r)
    xiT = nc.dram_tensor("xiT", [DI, NI], f32r)
    tpT = nc.dram_tensor("tpT", [D, NT], f32r)   # text_proj^T
    ipT = nc.dram_tensor("ipT", [D, NI], f32r)   # image_proj^T
    v_ta = nc.dram_tensor("v_ta", [bc, LT, D], f32r)       # per-batch V (text keys)
    v_ia = nc.dram_tensor("v_ia", [bc // 2, 2 * LI, D], f32r)  # batch-pair V
    att1tT = nc.dram_tensor("att1tT", [D, NI], f32r)  # ln(cross text_att)^T
    att1iT = nc.dram_tensor("att1iT", [D, NT], f32r)  # ln(cross image_att)^T
    v_ts = nc.dram_tensor("v_ts", [bc // 2, 2 * LI, D], f32r)
    v_is = nc.dram_tensor("v_is", [bc, LT, D], f32r)

    with tile.TileContext(nc) as tc:
        # ---------- persistent small constants ----------
        const_pool = tc.tile_pool(name="consts", bufs=1)
        cp = const_pool.__enter__()
        ident_f = cp.tile([128, 128], f32, tag="idf")
        make_identity(nc, ident_f)
        ident = cp.tile([128, 128], f32r, tag="idr")
        nc.vector.tensor_copy(ident[:], ident_f[:])
        ones_row_f = cp.tile([1, 128], f32, tag="o1rf")
        nc.vector.memset(ones_row_f[:], 1.0)
        ones_row = cp.tile([1, 128], f32r, tag="o1r")
        nc.vector.tensor_copy(ones_row[:], ones_row_f[:])
        ones_col_f = cp.tile([128, 1], f32, tag="o1cf")
        nc.vector.memset(ones_col_f[:], 1.0)
        ones_col = cp.tile([128, 1], f32r, tag="o1c")
        nc.vector.tensor_copy(ones_col[:], ones_col_f[:])
        eps_col = cp.tile([128, 1], f32, tag="epsc")
        nc.vector.memset(eps_col[:], EPS)
        sel_f = cp.tile([1, 256], f32, tag="self")
        nc.vector.memset(sel_f[:], 0.0)
        nc.vector.memset(sel_f[:, 0:64], 1.0)
        nc.vector.memset(sel_f[:, 192:256], 1.0)
        sel = cp.tile([1, 256], f32r, tag="selr")
        nc.vector.tensor_copy(sel[:], sel_f[:])
        sel0 = sel[:, 0:128]
        sel1 = sel[:, 128:256]

        def load_pc_vec(pool, dram_vec, tag):
            """[D] f32 dram -> [128, D//128] sbuf (feature chunk layout)."""
            t = pool.tile([128, D // 128], f32, tag=tag)
            with nc.allow_non_contiguous_dma(reason="tiny aux vector"):
                nc.sync.dma_start(t[:], dram_vec.rearrange("(c p) -> p c", p=128))
            return t

        # =========================================================
        # Phase 0: transpose raw embeddings -> xtT, xiT  (feature-major)
        # =========================================================
        def transpose_to_dram(src, srcT, ntok, dfeat):
            with tc.tile_pool(name="tr_in", bufs=3) as tin, \
                 tc.tile_pool(name="tr_out", bufs=4) as tout, \
                 tc.tile_pool(name="tr_ps", bufs=4, space="PSUM") as tps:
                for to in range(ntok // 128):
                    it = tin.tile([128, dfeat], f32r, tag="in")
                    nc.sync.dma_start(it[:], src[to * 128:(to + 1) * 128, :])
                    for fo in range(dfeat // 128):
                        ps = tps.tile([128, 128], f32r, tag="ps")
                        nc.tensor.transpose(ps[:], it[:, fo * 128:(fo + 1) * 128],
                                            ident[:])
                        ot = tout.tile([128, 128], f32r, tag="ot")
                        nc.vector.tensor_copy(ot[:], ps[:])
                        nc.sync.dma_start(
                            srcT[fo * 128:(fo + 1) * 128,
                                 to * 128:(to + 1) * 128], ot[:])

        transpose_to_dram(xt, xtT, NT, DT)
        transpose_to_dram(xi, xiT, NI, DI)

        # =========================================================
        # generic projections
        # =========================================================
        def proj_featmaj(w_dram, din, dout, src_rhs, ntok, out_write,
                         bias_sb=None):
            """out^T[dout, ntok] = W^T @ X^T. src_rhs(ko, no, nsz) -> rhs AP
            [128, nsz]; out_write(mo, no, nsz, psum_ap) emits copyback."""
            KC = din // 128
            MC = dout // 128
            NO = (ntok + 511) // 512
            with tc.tile_pool(name="pw", bufs=1) as pw, \
                 tc.tile_pool(name="ppsum", bufs=4, space="PSUM") as pp:
                w_sb = pw.tile([128, KC, dout], f32r, tag="w")
                nc.sync.dma_start(
                    w_sb[:], w_dram.rearrange("(kc p) n -> p kc n", p=128))
                for no in range(NO):
                    nsz = min(512, ntok - no * 512)
                    rhs = [src_rhs(ko, no, nsz) for ko in range(KC)]
                    for mo in range(MC):
                        ps = pp.tile([128, 512], f32, tag="ps")
                        for ko in range(KC):
                            nc.tensor.matmul(
                                ps[:, :nsz],
                                w_sb[:, ko, mo * 128:(mo + 1) * 128],
                                rhs[ko][:, :nsz],
                                start=(ko == 0), stop=(ko == KC - 1))
                        out_write(mo, no, nsz, ps)

        def dram_rhs_loader(pool, srcT, tag):
            def f(ko, no, nsz):
                t = pool.tile([128, 512], f32r, tag=tag)
                nc.sync.dma_start(t[:, :nsz],
                                  srcT[ko * 128:(ko + 1) * 128,
                                       no * 512:no * 512 + nsz])
                return t
            return f

        def proj_tokmaj(w_dram, din, dout, lhsT_src, tok_tiles, out_write):
            """out[tok, dout] = X @ W. lhsT_src(ti, ko) -> [128, rows<=128]
            stationary AP; out_write(ti, no, nsz, rows, psum)."""
            KC = din // 128
            NO = dout // 512
            with tc.tile_pool(name="pw2", bufs=1) as pw, \
                 tc.tile_pool(name="ppsum2", bufs=4, space="PSUM") as pp:
                w_sb = pw.tile([128, KC, dout], f32r, tag="w")
                nc.sync.dma_start(
                    w_sb[:], w_dram.rearrange("(kc p) n -> p kc n", p=128))
                for ti, rows in tok_tiles:
                    for no in range(NO):
                        nsz = 512
                        ps = pp.tile([128, 512], f32, tag="ps")
                        for ko in range(KC):
                            nc.tensor.matmul(
                                ps[:rows, :nsz],
                                lhsT_src(ti, ko),
                                w_sb[:, ko, no * 512:no * 512 + nsz],
                                start=(ko == 0), stop=(ko == KC - 1))
                        out_write(ti, no, nsz, rows, ps)

        # =========================================================
        # Phase 1: text_proj^T, image_proj^T  (with bias), into DRAM
        # =========================================================
        with tc.tile_pool(name="p1aux", bufs=1) as p1aux, \
             tc.tile_pool(name="p1rhs", bufs=18) as p1rhs, \
             tc.tile_pool(name="p1out", bufs=4) as p1out:
            btp = load_pc_vec(p1aux, V1["b_tp"], "btp")

            def wr_tp(mo, no, nsz, ps):
                o = p1out.tile([128, 512], f32r, tag="o")
                nc.vector.tensor_scalar(o[:, :nsz], ps[:, :nsz],
                                        btp[:, mo:mo + 1], None, ALU.add)
                nc.sync.dma_start(
                    tpT[mo * 128:(mo + 1) * 128, no * 512:no * 512 + nsz],
                    o[:, :nsz])
            proj_featmaj(W["w_tp"], DT, D, dram_rhs_loader(p1rhs, xtT, "x"),
                         NT, wr_tp)

            bip = load_pc_vec(p1aux, V1["b_ip"], "bip")

            def wr_ip(mo, no, nsz, ps):
                o = p1out.tile([128, 512], f32r, tag="o")
                nc.vector.tensor_scalar(o[:, :nsz], ps[:, :nsz],
                                        bip[:, mo:mo + 1], None, ALU.add)
                nc.sync.dma_start(
                    ipT[mo * 128:(mo + 1) * 128, no * 512:no * 512 + nsz],
                    o[:, :nsz])
            proj_featmaj(W["w_ip"], DI, D, dram_rhs_loader(p1rhs, xiT, "x2"),
                         NI, wr_ip)

        # =========================================================
        # Phase 2: value projections (token-major, per-batch tiles)
        # =========================================================
        def v_proj(w_dram, din, xT_dram, vd, tiles_spec):
            # tiles_spec: list of (ti, rows, colslice_start)
            with tc.tile_pool(name="vx", bufs=4) as vx, \
                 tc.tile_pool(name="vo", bufs=4) as vo:
                KC = din // 128
                xcols = {}

                def lhsT_src(ti, ko):
                    if ti not in xcols:
                        rows = tiles_spec[ti][1]
                        cs = tiles_spec[ti][2]
                        t = vx.tile([128, KC, 128], f32r, tag="xc")
                        nc.sync.dma_start(
                            t[:, :, :rows],
                            xT_dram[:, cs:cs + rows].rearrange(
                                "(kc p) t -> p kc t", p=128))
                        xcols[ti] = t
                        if len(xcols) > 3:
                            xcols.pop(next(iter(xcols)))
                    rows = tiles_spec[ti][1]
                    return xcols[ti][:, ko, :rows]

                def out_write(ti, no, nsz, rows, ps):
                    o = vo.tile([128, 512], f32r, tag="o")
                    nc.vector.tensor_copy(o[:rows, :nsz], ps[:rows, :nsz])
                    nc.sync.dma_start(
                        vd[ti, 0:rows, no * 512:no * 512 + nsz],
                        o[:rows, :nsz])

                proj_tokmaj(w_dram, din, D, lhsT_src,
                            [(ti, rows) for ti, rows, _ in tiles_spec],
                            out_write)

        v_proj(W["ta_wv"], DT, xtT, v_ta,
               [(b, LT, b * LT) for b in range(bc)])
        v_proj(W["ia_wv"], DI, xiT, v_ia,
               [(p, 128, p * 128) for p in range(bc // 2)])

        # =========================================================
        # attention block
        # =========================================================
        def attention(qT_sb, kT_sb, vd, Lq, Lk, pair_mode, am_dram, rv_dram,
                      oT_sb, ntokq):
            """qT_sb/kT_sb: [128, KD, ntok] sbuf; vd: dram V; oT_sb out."""
            with tc.tile_pool(name="at_v", bufs=3) as atv, \
                 tc.tile_pool(name="at_am", bufs=4) as atam, \
                 tc.tile_pool(name="at_sm", bufs=6) as atsm, \
                 tc.tile_pool(name="at_aT", bufs=4) as ataT, \
                 tc.tile_pool(name="at_ps", bufs=2, space="PSUM") as sps, \
                 tc.tile_pool(name="at_pt", bufs=2, space="PSUM") as tps, \
                 tc.tile_pool(name="at_pv", bufs=4, space="PSUM") as vps, \
                 tc.tile_pool(name="at_rv", bufs=1) as atrv:
                rvt = atrv.tile([128, bc], f32, tag="rv")
                with nc.allow_non_contiguous_dma(reason="tiny rowvalid"):
                    nc.sync.dma_start(rvt[:Lq, :],
                                      rv_dram.rearrange("b q -> q b"))
                vrows = 2 * Lk if pair_mode else Lk
                for b in range(bc):
                    if pair_mode:
                        if b % 2 == 0:
                            vtile = atv.tile([128, D], f32r, tag="v")
                            nc.sync.dma_start(vtile[:], vd[b // 2])
                        vbase = (b % 2) * Lk
                    else:
                        vtile = atv.tile([Lk, D], f32r, tag="v")
                        nc.sync.dma_start(vtile[:], vd[b])
                        vbase = 0
                    amr = atam.tile([1, Lk], f32r, tag="am")
                    nc.sync.dma_start(amr[:], am_dram[b:b + 1, :])
                    for h in range(H):
                        S = sps.tile([Lq, Lk], f32, tag="S")
                        for c in range(2):
                            nc.tensor.matmul(
                                S[:],
                                qT_sb[:, 2 * h + c, b * Lq:(b + 1) * Lq],
                                kT_sb[:, 2 * h + c, b * Lk:(b + 1) * Lk],
                                start=(c == 0), stop=False)
                        nc.tensor.matmul(S[:], ones_row[:, :Lq], amr[:],
                                         start=False, stop=True)
                        negmax = atsm.tile([Lq, 1], f32, tag="nm")
                        nc.vector.reduce_max(negmax[:], S[:], AX)
                        nc.vector.tensor_scalar_mul(negmax[:], negmax[:],
                                                    -1.0 / 16.0)
                        P = atsm.tile([Lq, Lk], f32, tag="P")
                        denom = atsm.tile([Lq, 1], f32, tag="dn")
                        nc.scalar.activation(P[:], S[:], AF.Exp,
                                             bias=negmax[:], scale=1.0 / 16.0,
                                             accum_out=denom[:])
                        rsc = atsm.tile([Lq, 1], f32, tag="rs")
                        nc.vector.reciprocal(rsc[:], denom[:])
                        nc.vector.tensor_mul(rsc[:], rsc[:],
                                             rvt[:Lq, b:b + 1])
                        Pn = atsm.tile([Lq, Lk], f32r, tag="Pn")
                        nc.vector.tensor_scalar_mul(Pn[:], P[:], rsc[:])
                        aTp = tps.tile([Lk, Lq], f32r, tag="aT")
                        nc.tensor.transpose(aTp[:], Pn[:], ident[:Lq, :Lq])
                        aT = ataT.tile([128, Lq], f32r, tag="aTs")
                        nc.vector.tensor_copy(aT[vbase:vbase + Lk, :], aTp[:])
                        for dvs in range(2):
                            fo = 2 * h + dvs
                            pv = vps.tile([128, Lq], f32, tag="pv")
                            nc.tensor.matmul(
                                pv[:],
                                vtile[vbase:vbase + Lk,
                                      fo * 128:(fo + 1) * 128],
                                aT[vbase:vbase + Lk, :],
                                start=True, stop=True)
                            nc.vector.tensor_copy(
                                oT_sb[:, fo, b * Lq:(b + 1) * Lq], pv[:])

        def attention_packed(qT_sb, kT_sb, vd, Lk, pair_v, am_dram, rv_dram,
                             oT_sb):
            """Lq=64 attentions: two batches per tile (block-diagonal S).
            vd: pair tiles [128, D] if pair_v else per-batch [Lk, D]."""
            Lk2 = 2 * Lk
            with tc.tile_pool(name="ap_v", bufs=3) as atv, \
                 tc.tile_pool(name="ap_am", bufs=4) as atam, \
                 tc.tile_pool(name="ap_sm", bufs=8) as atsm, \
                 tc.tile_pool(name="ap_aT", bufs=4) as ataT, \
                 tc.tile_pool(name="ap_ps", bufs=3, space="PSUM") as sps, \
                 tc.tile_pool(name="ap_pt", bufs=2, space="PSUM") as tps, \
                 tc.tile_pool(name="ap_pv", bufs=2, space="PSUM") as vps, \
                 tc.tile_pool(name="ap_rv", bufs=1) as atrv:
                rvt = atrv.tile([128, bc // 2], f32, tag="rv")
                with nc.allow_non_contiguous_dma(reason="tiny rowvalid"):
                    nc.sync.dma_start(
                        rvt[:], rv_dram.rearrange("(p two) q -> (two q) p",
                                                  two=2))
                for p in range(bc // 2):
                    b0, b1 = 2 * p, 2 * p + 1
                    v0 = atv.tile([Lk, D], f32r, tag="v")
                    v1 = atv.tile([Lk, D], f32r, tag="v")
                    if pair_v:
                        nc.sync.dma_start(v0[:], vd[p, 0:Lk])
                        nc.sync.dma_start(v1[:], vd[p, Lk:2 * Lk])
                    else:
                        nc.sync.dma_start(v0[:], vd[b0])
                        nc.sync.dma_start(v1[:], vd[b1])
                    amp0f = atam.tile([1, Lk2], f32, tag="am0f")
                    nc.vector.memset(amp0f[:], NEGBIG)
                    nc.sync.dma_start(amp0f[:, 0:Lk],
                                      am_dram[b0:b0 + 1, :].bitcast(f32))
                    amp0 = atam.tile([1, Lk2], f32r, tag="am0")
                    nc.vector.tensor_copy(amp0[:], amp0f[:])
                    amp1f = atam.tile([1, Lk2], f32, tag="am1f")
                    nc.vector.memset(amp1f[:], NEGBIG)
                    nc.sync.dma_start(amp1f[:, Lk:Lk2],
                                      am_dram[b1:b1 + 1, :].bitcast(f32))
                    amp1 = atam.tile([1, Lk2], f32r, tag="am1")
                    nc.vector.tensor_copy(amp1[:], amp1f[:])
                    for h in range(H):
                        S = sps.tile([128, Lk2], f32, tag="S")
                        for c in range(2):
                            nc.tensor.matmul(
                                S[:],
                                qT_sb[:, 2 * h + c, p * 128:(p + 1) * 128],
                                kT_sb[:, 2 * h + c, p * Lk2:(p + 1) * Lk2],
                                start=(c == 0), stop=False)
                        nc.tensor.matmul(S[:], sel0, amp0[:],
                                         start=False, stop=False)
                        nc.tensor.matmul(S[:], sel1, amp1[:],
                                         start=False, stop=True)
                        negmax = atsm.tile([128, 1], f32, tag="nm")
                        nc.vector.reduce_max(negmax[:], S[:], AX)
                        nc.vector.tensor_scalar_mul(negmax[:], negmax[:],
                                                    -1.0 / 16.0)
                        P = atsm.tile([128, Lk2], f32, tag="P")
                        denom = atsm.tile([128, 1], f32, tag="dn")
                        nc.scalar.activation(P[:], S[:], AF.Exp,
                                             bias=negmax[:], scale=1.0 / 16.0,
                                             accum_out=denom[:])
                        rsc = atsm.tile([128, 1], f32, tag="rs")
                        nc.vector.reciprocal(rsc[:], denom[:])
                        nc.vector.tensor_mul(rsc[:], rsc[:],
                                             rvt[:, p:p + 1])
                        Pn = atsm.tile([128, Lk2], f32r, tag="Pn")
                        nc.vector.tensor_scalar_mul(Pn[:], P[:], rsc[:])
                        aTp0 = tps.tile([Lk, 128], f32r, tag="aT0")
                        nc.tensor.transpose(aTp0[:], Pn[:, 0:Lk], ident[:])
                        aTp1 = tps.tile([Lk, 128], f32r, tag="aT0")
                        nc.tensor.transpose(aTp1[:], Pn[:, Lk:Lk2], ident[:])
                        aT0 = ataT.tile([Lk, 128], f32r, tag="aTs")
                        nc.vector.tensor_copy(aT0[:], aTp0[:])
                        aT1 = ataT.tile([Lk, 128], f32r, tag="aTs")
                        nc.vector.tensor_copy(aT1[:], aTp1[:])
                        r0, r1 = aT0[:], aT1[:]
                        for dvs in range(2):
                            fo = 2 * h + dvs
                            pv = vps.tile([128, 128], f32, tag="pv")
                            nc.tensor.matmul(
                                pv[:], v0[:, fo * 128:(fo + 1) * 128], r0,
                                start=True, stop=False)
                            nc.tensor.matmul(
                                pv[:], v1[:, fo * 128:(fo + 1) * 128], r1,
                                start=False, stop=True)
                            nc.vector.tensor_copy(
                                oT_sb[:, fo, p * 128:(p + 1) * 128], pv[:])

        # =========================================================
        # LN helpers
        # =========================================================
        def ln_featmaj(x_sb, ntok, g_sb, b_sb, outT_dram):
            """x_sb: [128, KD, ntok] f32r feature-major (raw).
            Writes normalized f32r to outT_dram."""
            with tc.tile_pool(name="lnsq", bufs=3) as lsq, \
                 tc.tile_pool(name="lnst", bufs=4) as lst, \
                 tc.tile_pool(name="lnps", bufs=2, space="PSUM") as lps, \
                 tc.tile_pool(name="lnrep", bufs=2, space="PSUM") as lrep, \
                 tc.tile_pool(name="lnout", bufs=4) as lout:
                NO = (ntok + 511) // 512
                for no in range(NO):
                    nsz = min(512, ntok - no * 512)
                    sl = slice(no * 512, no * 512 + nsz)
                    psum_t = lps.tile([1, 512], f32, tag="s")
                    psq = lps.tile([1, 512], f32, tag="q")
                    for ko in range(KD):
                        nc.tensor.matmul(psum_t[:, :nsz], ones_col[:],
                                         x_sb[:, ko, sl],
                                         start=(ko == 0), stop=(ko == KD - 1))
                    sqt = None
                    for ko in range(KD):
                        sq = lsq.tile([128, 512], f32r, tag="sq")
                        nc.vector.tensor_mul(sq[:, :nsz], x_sb[:, ko, sl],
                                             x_sb[:, ko, sl])
                        nc.tensor.matmul(psq[:, :nsz], ones_col[:],
                                         sq[:, :nsz],
                                         start=(ko == 0), stop=(ko == KD - 1))
                    stats = lst.tile([1, 1024], f32r, tag="st")
                    mean = lst.tile([1, 512], f32, tag="mn")
                    nc.vector.tensor_scalar_mul(mean[:, :nsz], psum_t[:, :nsz],
                                                1.0 / D)
                    ex2 = lst.tile([1, 512], f32, tag="e2")
                    nc.vector.tensor_scalar_mul(ex2[:, :nsz], psq[:, :nsz],
                                                1.0 / D)
                    msq = lst.tile([1, 512], f32, tag="m2")
                    nc.vector.tensor_mul(msq[:, :nsz], mean[:, :nsz],
                                         mean[:, :nsz])
                    var = lst.tile([1, 512], f32, tag="vr")
                    nc.vector.tensor_sub(var[:, :nsz], ex2[:, :nsz],
                                         msq[:, :nsz])
                    # stats[0:512] = rstd, stats[512:1024] = mean*rstd
                    sd = lst.tile([1, 512], f32, tag="sd")
                    nc.scalar.activation(sd[:, :nsz], var[:, :nsz],
                                         AF.Sqrt, bias=eps_col[:1], scale=1.0)
                    with nc.allow_low_precision(reason="f32r bytes==f32"):
                        nc.vector.reciprocal(stats[:, :nsz], sd[:, :nsz])
                    nc.vector.tensor_mul(stats[:, 512:512 + nsz],
                                         mean[:, :nsz], stats[:, :nsz])
                    rep_r = lrep.tile([128, 512], f32, tag="rr")
                    nc.tensor.matmul(rep_r[:, :nsz], ones_row[:, :],
                                     stats[:, :nsz], start=True, stop=True)
                    rep_m = lrep.tile([128, 512], f32, tag="rm")
                    nc.tensor.matmul(rep_m[:, :nsz], ones_row[:, :],
                                     stats[:, 512:512 + nsz],
                                     start=True, stop=True)
                    for ko in range(KD):
                        o = lout.tile([128, 512], f32r, tag="o")
                        nc.vector.tensor_mul(o[:, :nsz], x_sb[:, ko, sl],
                                             rep_r[:, :nsz])
                        nc.vector.tensor_sub(o[:, :nsz], o[:, :nsz],
                                             rep_m[:, :nsz])
                        nc.vector.tensor_scalar(o[:, :nsz], o[:, :nsz],
                                                g_sb[:, ko:ko + 1],
                                                b_sb[:, ko:ko + 1],
                                                ALU.mult, ALU.add)
                        nc.sync.dma_start(
                            outT_dram[ko * 128:(ko + 1) * 128, sl],
                            o[:, :nsz])

        def ln_tokmaj_out(ps_getter, tok_tiles, g_sb_row, b_sb_row, out_dram):
            """Final LN on token-major tiles; ps_getter(ti) -> sbuf [rows, D]
            f32 tile. Writes f32 out_dram rows."""
            with tc.tile_pool(name="lnf", bufs=4) as lnf:
                for ti, rows, rstart in tok_tiles:
                    x = ps_getter(ti, rows)
                    mean = lnf.tile([128, 1], f32, tag="mn")
                    nc.vector.reduce_sum(mean[:rows], x[:rows], AX)
                    nc.vector.tensor_scalar_mul(mean[:rows], mean[:rows],
                                                1.0 / D)
                    nc.vector.tensor_scalar(x[:rows], x[:rows], mean[:rows],
                                            None, ALU.subtract)
                    sq = lnf.tile([128, D], f32, tag="sq")
                    nc.vector.tensor_mul(sq[:rows], x[:rows], x[:rows])
                    vs = lnf.tile([128, 1], f32, tag="vs")
                    nc.vector.reduce_sum(vs[:rows], sq[:rows], AX)
                    rstd = lnf.tile([128, 1], f32, tag="rs")
                    nc.scalar.activation(rstd[:rows], vs[:rows], AF.Sqrt,
                                         bias=eps_col[:rows], scale=1.0 / D)
                    nc.vector.reciprocal(rstd[:rows], rstd[:rows])
                    nc.vector.tensor_scalar_mul(x[:rows], x[:rows],
                                                rstd[:rows])
                    o = lnf.tile([128, D], f32, tag="o")
                    nc.vector.tensor_mul(o[:rows], x[:rows], g_sb_row[:rows])
                    nc.vector.tensor_add(o[:rows], o[:rows], b_sb_row[:rows])
                    nc.sync.dma_start(out_dram[rstart:rstart + rows, :],
                                      o[:rows])

        # =========================================================
        # cross-attention pipelines
        # =========================================================
        def cross_side(wq, wk, wo, gname, bname, qsrcT, ksrcT, vd, Lq, Lk,
                       pair_mode, am_dram, rv_dram, ntokq, outT_dram):
            po_cm = tc.tile_pool(name="cs_o", bufs=1)
            po = po_cm.__enter__()
            pq_cm = tc.tile_pool(name="cs_q", bufs=1)
            pq = pq_cm.__enter__()
            qT = pq.tile([128, KD, ntokq], f32r, tag="qT")
            kT = pq.tile([128, KD, bc * Lk], f32r, tag="kT")

            prhs_cm = tc.tile_pool(name="cs_rhs", bufs=10)
            prhs = prhs_cm.__enter__()

            def wr_q(mo, no, nsz, ps):
                nc.vector.tensor_copy(
                    qT[:, mo, no * 512:no * 512 + nsz], ps[:, :nsz])
            proj_featmaj(W[wq], D, D, dram_rhs_loader(prhs, qsrcT, "xc"),
                         ntokq, wr_q)

            def wr_k(mo, no, nsz, ps):
                nc.vector.tensor_copy(
                    kT[:, mo, no * 512:no * 512 + nsz], ps[:, :nsz])
            proj_featmaj(W[wk], D, D, dram_rhs_loader(prhs, ksrcT, "xc"),
                         bc * Lk, wr_k)
            prhs_cm.__exit__(None, None, None)

            oT = po.tile([128, KD, ntokq], f32r, tag="oT")
            if Lq == 64 and (PACK_MODE & 1):
                attention_packed(qT, kT, vd, Lk, False, am_dram, rv_dram, oT)
            else:
                attention(qT, kT, vd, Lq, Lk, pair_mode, am_dram,
                          rv_dram, oT, ntokq)
            pq_cm.__exit__(None, None, None)

            # Wo projection (feature-major out) + LN -> DRAM
            with tc.tile_pool(name="cs_w1", bufs=1) as pw1, \
                 tc.tile_pool(name="cs_ln", bufs=1) as pln:
                araw = pw1.tile([128, KD, ntokq], f32r, tag="ar")

                def wr_a(mo, no, nsz, ps):
                    nc.vector.tensor_copy(
                        araw[:, mo, no * 512:no * 512 + nsz],
                        ps[:, :nsz])

                def orhs(ko, no, nsz):
                    return oT[:, ko, no * 512:no * 512 + nsz]
                proj_featmaj(W[wo], D, D, orhs, ntokq, wr_a)
                g_sb = load_pc_vec(pln, V1[gname], "g")
                b_sb = load_pc_vec(pln, V1[bname], "b")
                ln_featmaj(araw, ntokq, g_sb, b_sb, outT_dram)
            po_cm.__exit__(None, None, None)

        # ta: queries = image side (Lq=LI), keys/values = text side
        cross_side("ta_wq", "ta_wk", "ta_wo", "g_ta", "bt_ta",
                   ipT, tpT, v_ta, LI, LT, False, am_t, rv_ta, NI, att1tT)
        # ia: queries = text side, keys/values = image side (pair mode)
        cross_side("ia_wq", "ia_wk", "ia_wo", "g_ia", "bt_ia",
                   tpT, ipT, v_ia, LT, LI, True, am_i, rv_ia, NT, att1iT)

        # =========================================================
        # self-attention V projections (Q/K projected inside self_side)
        # =========================================================
        v_proj(W["ts_wv"], D, att1tT, v_ts,
               [(p, 128, p * 128) for p in range(bc // 2)])
        v_proj(W["is_wv"], D, att1iT, v_is,
               [(b, LT, b * LT) for b in range(bc)])

        # =========================================================
        # self-attention + final Wo (token-major) + final LN -> outputs
        # =========================================================
        def self_side(qsrcT, vd, Lqk, pair_mode, am_dram, rv_dram, ntok,
                      wq, wk, wo, gname, bname, out_dram):
            po_cm = tc.tile_pool(name="ss_o", bufs=1)
            po = po_cm.__enter__()
            pq_cm = tc.tile_pool(name="ss_q", bufs=1)
            pq = pq_cm.__enter__()
            qT = pq.tile([128, KD, ntok], f32r, tag="qT")
            kT = pq.tile([128, KD, ntok], f32r, tag="kT")
            prhs_cm = tc.tile_pool(name="ss_rhs", bufs=10)
            prhs = prhs_cm.__enter__()

            def wr_q(mo, no, nsz, ps):
                nc.vector.tensor_copy(
                    qT[:, mo, no * 512:no * 512 + nsz], ps[:, :nsz])
            proj_featmaj(W[wq], D, D, dram_rhs_loader(prhs, qsrcT, "xs"),
                         ntok, wr_q)

            def wr_k(mo, no, nsz, ps):
                nc.vector.tensor_copy(
                    kT[:, mo, no * 512:no * 512 + nsz], ps[:, :nsz])
            proj_featmaj(W[wk], D, D, dram_rhs_loader(prhs, qsrcT, "xs"),
                         ntok, wr_k)
            prhs_cm.__exit__(None, None, None)

            oT = po.tile([128, KD, ntok], f32r, tag="oT")
            if Lqk == 64 and (PACK_MODE & 2):
                attention_packed(qT, kT, vd, Lqk, True, am_dram, rv_dram, oT)
            else:
                attention(qT, kT, vd, Lqk, Lqk, pair_mode, am_dram,
                          rv_dram, oT, ntok)
            pq_cm.__exit__(None, None, None)

            # final Wo token-major + LN
            with tc.tile_pool(name="ss_w", bufs=1) as pw, \
                 tc.tile_pool(name="ss_x", bufs=3) as px, \
                 tc.tile_pool(name="ss_ps", bufs=4,
                              space="PSUM") as pp, \
                 tc.tile_pool(name="ss_ln", bufs=1) as pln:
                w_sb = pw.tile([128, KD, D], f32r, tag="w")
                nc.sync.dma_start(
                    w_sb[:],
                    W[wo].rearrange("(kc p) n -> p kc n", p=128))
                # replicate g/b rows across partitions via PE
                grow = pln.tile([1, D], f32r, tag="gr")
                brow = pln.tile([1, D], f32r, tag="br")
                nc.sync.dma_start(grow[:], V1[gname][None, :].bitcast(f32r))
                nc.sync.dma_start(brow[:], V1[bname][None, :].bitcast(f32r))
                g_rep = pln.tile([128, D], f32, tag="grep")
                b_rep = pln.tile([128, D], f32, tag="brep")
                for half in range(2):
                    hs = slice(half * 512, (half + 1) * 512)
                    rp = pp.tile([128, 512], f32, tag="rep")
                    nc.tensor.matmul(rp[:], ones_row[:], grow[:, hs],
                                     start=True, stop=True)
                    nc.vector.tensor_copy(g_rep[:, hs], rp[:])
                    rp2 = pp.tile([128, 512], f32, tag="rep")
                    nc.tensor.matmul(rp2[:], ones_row[:], brow[:, hs],
                                     start=True, stop=True)
                    nc.vector.tensor_copy(b_rep[:, hs], rp2[:])

                x_tiles = {}

                def getx(ti, rows):
                    return x_tiles.pop(ti)

                for ti in range(ntok // 128):
                    xsb = px.tile([128, D], f32, tag="x")
                    for no in range(2):
                        ps = pp.tile([128, 512], f32, tag="ps")
                        for ko in range(KD):
                            nc.tensor.matmul(
                                ps[:],
                                oT[:, ko, ti * 128:(ti + 1) * 128],
                                w_sb[:, ko, no * 512:(no + 1) * 512],
                                start=(ko == 0), stop=(ko == KD - 1))
                        nc.vector.tensor_copy(
                            xsb[:, no * 512:(no + 1) * 512], ps[:])
                    x_tiles[ti] = xsb
                    ln_tokmaj_out(getx,
                                  [(ti, 128, ti * 128)],
                                  g_rep, b_rep, out_dram)
            po_cm.__exit__(None, None, None)

        self_side(att1tT, v_ts, LI, True, am_i, rv_ts, NI,
                  "ts_wq", "ts_wk", "ts_wo", "g_ts", "bt_ts", out_t)
        self_side(att1iT, v_is, LT, False, am_t, rv_is, NT,
                  "is_wq", "is_wk", "is_wo", "g_is", "bt_is", out_i)

        const_pool.__exit__(None, None, None)

    nc.compile()
    return nc


def _get_nc(bc):
    if bc not in _CACHE:
        _CACHE[bc] = _build(bc)
    return _CACHE[bc]


def kernel(text_embedding, image_embedding, text_mask, image_mask,
           W_tp, b_tp, W_ip, b_ip,
           ta_Wq, ta_Wk, ta_Wv, ta_Wo,
           ia_Wq, ia_Wk, ia_Wv, ia_Wo,
           ts_Wq, ts_Wk, ts_Wv, ts_Wo,
           is_Wq, is_Wk, is_Wv, is_Wo,
           ln_ta_g, ln_ta_b, ln_ia_g, ln_ia_b,
           ln_ts_g, ln_ts_b, ln_is_g, ln_is_b):
    from concourse.bass_utils import run_bass_kernel_spmd

    bc = B // NCORES
    nc = _get_nc(bc)

    f = np.float32
    wmap = {
        "w_tp": W_tp, "w_ip": W_ip,
        "ta_wq": ta_Wq, "ta_wk": ta_Wk, "ta_wv": ta_Wv, "ta_wo": ta_Wo,
        "ia_wq": ia_Wq, "ia_wk": ia_Wk, "ia_wv": ia_Wv, "ia_wo": ia_Wo,
        "ts_wq": ts_Wq, "ts_wk": ts_Wk, "ts_wv": ts_Wv, "ts_wo": ts_Wo,
        "is_wq": is_Wq, "is_wk": is_Wk, "is_wv": is_Wv, "is_wo": is_Wo,
        "b_tp": b_tp, "b_ip": b_ip,
        "g_ta": ln_ta_g, "bt_ta": ln_ta_b, "g_ia": ln_ia_g, "bt_ia": ln_ia_b,
        "g_ts": ln_ts_g, "bt_ts": ln_ts_b, "g_is": ln_is_g, "bt_is": ln_is_b,
    }
    wmap = {k: np.ascontiguousarray(np.asarray(v), dtype=f)
            for k, v in wmap.items()}

    in_maps = []
    for c in range(NCORES):
        sl = slice(c * bc, (c + 1) * bc)
        tm = np.asarray(text_mask[sl]).astype(f)
        im = np.asarray(image_mask[sl]).astype(f)
        m = dict(wmap)
        m["xt"] = np.ascontiguousarray(
            np.asarray(text_embedding[sl]), dtype=f).reshape(bc * LT, DT)
        m["xi"] = np.ascontiguousarray(
            np.asarray(image_embedding[sl]), dtype=f).reshape(bc * LI, DI)
        m["am_t"] = (NEGBIG * (1.0 - tm)).astype(f)
        m["am_i"] = (NEGBIG * (1.0 - im)).astype(f)
        any_t = (tm.max(axis=1) > 0).astype(f)[:, None]
        any_i = (im.max(axis=1) > 0).astype(f)[:, None]
        m["rv_ta"] = (im * any_t).astype(f)
        m["rv_ia"] = (tm * any_i).astype(f)
        m["rv_ts"] = (im * any_i).astype(f)
        m["rv_is"] = (tm * any_t).astype(f)
        in_maps.append(m)

    res = run_bass_kernel_spmd(nc, in_maps, list(range(NCORES)))
    text = np.concatenate(
        [res.results[c]["out_t"].reshape(bc, LI, D) for c in range(NCORES)],
        axis=0)
    image = np.concatenate(
        [res.results[c]["out_i"].reshape(bc, LT, D) for c in range(NCORES)],
        axis=0)
    return text, image
